# revision 14
# baseline (speedup 1.0000x reference)
"""Self-contained Trainium2 Bass kernel for sliding-window attention.

Problem (hardcoded): B=1, S=8192, dim=1024, H=16 heads, D=64 head dim,
window=512, fp32 I/O.  y = (softmax(mask(rope(xWq^T) rope(xWk^T)^T / 8)) xWv^T) Wo^T

Strategy: sequence-parallel over 8 NeuronCores. Each core owns 1024 query
rows and additionally recomputes K/V for the 512-row halo to its left
(core 0's halo is zero-padded and neutralized via a per-core "vones"
column so no collective is needed).  All matmuls run in bf16 (fp32 PSUM
accumulation).

v2 schedule changes vs baseline:
  - priority-ordered fine-grained input DMAs round-robined over 4 queues
    (first-needed tiles land first) + PE warm-up matmuls during the load
    to climb the tensor-engine p-state ramp.
  - rope uses a 16-wide even/odd interleave per head so the pair swap is
    a single DVE stream_shuffle (within-32-partition), plus a signed sin
    table: 1 shuffle + 2 muls + 1 add, all 128-partition-wide.
  - sliding-window causality applied as 0/1 mask multiplies on the DVE
    (post-exp) instead of rank-128 mask matmuls on the PE.
  - per-(head,kt) software pipelining: scores(kt+1) is emitted before
    exp/PV(kt) so the PE streams while the scalar engine runs exp.
  - softmax denominator broadcast via a stride-0 DMA instead of the slow
    gpsimd partition_broadcast.
  - output projection evictions alternate vector/scalar and out-tiles DMA
    immediately on rotating queues (no serial tail).

Layouts (per core):
  xT    [1024(d), 1536(s)]  x^T shard incl. halo (bf16)
  wq/wk [1024(d), 1024(e')] Wq^T / Wk^T with a per-head column permutation
                            (rope de-interleave: head h's rows are
                            [E0-15 | O0-15 | E16-31 | O16-31])
  wv    [1024(d), 1024(e)]  Wv^T (no permutation), wo = Wo^T
  Q^T/K^T are produced in [e', s] layout (weight-stationary matmuls) so
  attention needs no transposes: scores are computed transposed,
  S^T[k, q], the softmax denominator comes free from a ones-column
  appended to V, and PV directly yields o^T[e, q] — the lhsT of the
  output projection.
"""
import sys

sys.path.insert(0, "/opt/trn_rl_repo")

import numpy as np
import ml_dtypes

import concourse.bass as bass
import concourse.mybir as mybir
from concourse import bacc
from concourse.tile import TileContext
from concourse.bass_utils import run_bass_kernel_spmd

BF = ml_dtypes.bfloat16
NCORES = 8
S, DIM, H, D, W = 8192, 1024, 16, 64, 512
SL = S // NCORES          # 1024 own rows / core
SK = SL + W               # 1536 rows incl. left halo
P = 128
NKT = SK // P             # 12 kv tiles
NQB = SL // P             # 8 query tiles
dt = mybir.dt

_compiled = {}

SWAP16 = list(range(16, 32)) + list(range(16))  # within-32 E<->O block swap
VA = 80   # V_aug columns: 64 V + 1 ones + pad (32B-aligned stride)


def _build():
    nc = bacc.Bacc("TRN2", target_bir_lowering=False, debug=False,
                   num_devices=NCORES)
    def param(name, shape, dtype=dt.bfloat16, out=False):
        return nc.declare_dram_parameter(name, shape, dtype, isOutput=out)

    xt = param("xt", [DIM, SK])
    wq = param("wq", [DIM, DIM])
    wk = param("wk", [DIM, DIM])
    wv = param("wv", [DIM, DIM])
    wo = param("wo", [DIM, DIM])
    ropc = param("ropc", [P, SK])
    rops = param("rops", [P, SK])       # sign-folded sin table
    vone = param("vone", [P, NKT])      # host-expanded per-kt validity
    mko = param("mko", [P, P])          # keep-mask, oldest block  [k, q]
    mkd = param("mkd", [P, P])          # keep-mask, diagonal block [k, q]
    out = param("out", [SL, DIM], dt.float32, out=True)

    with TileContext(nc) as tc:
        _body(nc, tc, xt, wq, wk, wv, wo, ropc, rops, vone, mko, mkd, out)
    nc.compile()
    return nc


def _brd2(ap_slice, n):
    """Insert a stride-0 middle free dim of size n into a [p, c] AP."""
    return bass.AP(tensor=ap_slice.tensor, offset=ap_slice.offset,
                   ap=[ap_slice.ap[0], [0, n], ap_slice.ap[1]])


def _body(nc, tc, xt, wq, wk, wv, wo, ropc, rops, vone, mko, mkd, out):
    f32, bf16 = dt.float32, dt.bfloat16

    with tc.tile_pool(name="persist", bufs=1) as per:
        # long-lived SBUF tensors
        v_sb = per.tile([P, NKT, H, VA], bf16)    # V_aug: [V(64)|ones|pad]
        qt_sb = per.tile([P, 8, SL], bf16)        # Q^T (rope'd, sigma layout)
        kt_sb = per.tile([P, 8, SK], bf16)        # K^T
        ot_sb = per.tile([P, 8, SL], bf16)        # o^T (normalized)
        ropc_sb = per.tile([P, SK], bf16)
        rops_sb = per.tile([P, SK], bf16)
        mko_sb = per.tile([P, P], bf16)
        mkd_sb = per.tile([P, P], bf16)
        vone_sb = per.tile([P, NKT], bf16)

        with tc.tile_pool(name="xtp", bufs=1) as xtp, \
             tc.tile_pool(name="proj", bufs=2, space="PSUM") as projp, \
             tc.tile_pool(name="raw", bufs=3) as rawp, \
             tc.tile_pool(name="swp", bufs=3) as swp, \
             tc.tile_pool(name="mcp", bufs=2) as mcp, \
             tc.tile_pool(name="msp", bufs=2) as msp, \
             tc.tile_pool(name="wqk", bufs=1) as wqkp, \
             tc.tile_pool(name="pt", bufs=3) as ptp, \
             tc.tile_pool(name="st", bufs=2, space="PSUM") as stp, \
             tc.tile_pool(name="pv", bufs=2, space="PSUM") as pvp, \
             tc.tile_pool(name="osb", bufs=2) as osbp, \
             tc.tile_pool(name="eps", bufs=1) as epsp, \
             tc.tile_pool(name="bcp", bufs=2) as bcp:
            xt_sb = xtp.tile([P, 8, SK], bf16)
            wk_sb = wqkp.tile([P, 8, DIM], bf16)
            wq_sb = wqkp.tile([P, 8, DIM], bf16)
            wv_sb = wqkp.tile([P, 8, DIM], bf16)
            wo_sb = wqkp.tile([P, 8, DIM], bf16)

            xt_r = xt.ap().rearrange("(d p) s -> p d s", p=P)
            wk_r = wk.ap().rearrange("(d p) e -> p d e", p=P)
            wq_r = wq.ap().rearrange("(d p) e -> p d e", p=P)
            wv_r = wv.ap().rearrange("(d p) e -> p d e", p=P)
            wo_r = wo.ap().rearrange("(e p) n -> p e n", p=P)

            # ---- priority-ordered input DMAs, round-robin over 3 queues
            # (each queue drains in order, so each queue's prefix is the
            # first-needed data; ~0.65us issue cost per DMA dominates below
            # ~256KB, so pieces are kept large) ----
            dmas = [
                (ropc_sb[:, :], ropc[:, :]),
                (rops_sb[:, :], rops[:, :]),
                (mko_sb[:, :], mko[:, :]),
                (mkd_sb[:, :], mkd[:, :]),
                (vone_sb[:, :], vone[:, :]),
            ]
            for d in range(8):
                dmas.append((xt_sb[:, d, 0:768], xt_r[:, d, 0:768]))
            for d in range(8):
                dmas.append((wk_sb[:, d, :], wk_r[:, d, :]))
            for d in range(8):
                dmas.append((xt_sb[:, d, 768:1536], xt_r[:, d, 768:1536]))
            for d in range(8):
                dmas.append((wq_sb[:, d, :], wq_r[:, d, :]))
            for d in range(8):
                dmas.append((wv_sb[:, d, :], wv_r[:, d, :]))
            for d in range(8):
                dmas.append((wo_sb[:, d, :], wo_r[:, d, :]))
            DQ = [nc.sync, nc.scalar, nc.gpsimd]
            for i, (o_ap, i_ap) in enumerate(dmas):
                DQ[i % 3].dma_start(out=o_ap, in_=i_ap)

            # ---- PE warm-up: dummy matmuls on the rope table while the
            # real inputs stream in, to climb the p-state ramp ----
            warm_ps = projp.tile([P, 512], f32, name="warm", tag="ps")
            for _ in range(8):
                nc.tensor.matmul(warm_ps, lhsT=ropc_sb[:, 0:P],
                                 rhs=ropc_sb[:, 0:512], start=True, stop=True)

            # ones column of V_aug (per-core halo-validity mask)
            for kt_i in range(NKT):
                nc.vector.tensor_copy(
                    out=v_sb[:, kt_i, :, 64],
                    in_=_brd2(vone_sb[:, kt_i:kt_i + 1], H))

            def rope_chunk(raw, dst, c0):
                # raw [P, 512] bf16 -> dst [P, 512] (rope'd).
                # dst = raw*cos + swap16(raw)*sin_signed
                cseg = slice(c0, c0 + 512)
                rsw = swp.tile([P, 512], bf16, name="rsw", tag="rsw")
                mcos = mcp.tile([P, 512], bf16, name="mcos", tag="mcos")
                msw = msp.tile([P, 512], bf16, name="msw", tag="msw")
                nc.vector.stream_shuffle(rsw, raw, SWAP16)
                nc.vector.tensor_mul(mcos, raw, ropc_sb[:, cseg])
                nc.vector.tensor_mul(msw, rsw, rops_sb[:, cseg])
                nc.vector.tensor_add(dst, mcos, msw)

            def proj_rope_et(w_sb, dst, s0, et, mid_hook=None):
                # dst[:, et, s0:SK] = rope((W^T)^T @ xT[:, s0:SK])
                for si, sp in enumerate(range(s0, SK, 512)):
                    ps = projp.tile([P, 512], f32, name="ps", tag="ps")
                    for d in range(8):
                        nc.tensor.matmul(
                            ps,
                            lhsT=w_sb[:, d, et * P:(et + 1) * P],
                            rhs=xt_sb[:, d, sp:sp + 512],
                            start=(d == 0), stop=(d == 7))
                    if si == 0 and mid_hook is not None:
                        mid_hook()
                    raw = rawp.tile([P, 512], bf16, name="raw", tag="raw")
                    nc.scalar.copy(out=raw, in_=ps)
                    rope_chunk(raw, dst[:, et, sp - s0:sp - s0 + 512], sp)

            # ---- attention with cross-(head,kt) software pipelining ----
            pv_state = {}     # h -> [pv_t_g0, pv_t_g1]
            pend = []         # [(h, kt, st_ps, lo, hi)]
            step = [0]        # global post counter
            deferred = []     # [(due_step, fn)] late-emitted retire tails

            def emit_scores(h, kt):
                et, hr = h // 2, (h % 2) * 64
                lo, hi = max(kt - 4, 0), min(kt, 7)
                nqb = hi - lo + 1
                n0 = min(nqb, 4) * P
                kh = kt_sb[hr:hr + 64, et, kt * P:(kt + 1) * P]
                st_ps = stp.tile([P, 640], f32, name="st_ps")
                nc.tensor.matmul(
                    st_ps[:, 0:n0], lhsT=kh,
                    rhs=qt_sb[hr:hr + 64, et, lo * P:lo * P + n0],
                    start=True, stop=True)
                if nqb == 5:
                    nc.tensor.matmul(
                        st_ps[:, 512:640], lhsT=kh,
                        rhs=qt_sb[hr:hr + 64, et, (lo + 4) * P:(lo + 5) * P],
                        start=True, stop=True)
                pend.append((h, kt, st_ps, lo, hi))

            def emit_post():
                if not pend:
                    return
                h, kt, st_ps, lo, hi = pend.pop(0)
                et, hr = h // 2, (h % 2) * 64
                nqb = hi - lo + 1
                pv_t = pv_state.setdefault(h, [None, None])
                p_t = ptp.tile([P, 640], bf16, name="p_t")
                nc.scalar.activation(
                    out=p_t[:, 0:nqb * P], in_=st_ps[:, 0:nqb * P],
                    func=mybir.ActivationFunctionType.Exp, scale=0.125)
                if kt >= 4:          # causal (diagonal) block sits at col 0
                    nc.vector.tensor_mul(p_t[:, 0:P], p_t[:, 0:P], mkd_sb)
                if kt <= 7:          # oldest block sits at col kt-lo
                    c = (kt - lo) * P
                    nc.vector.tensor_mul(p_t[:, c:c + P], p_t[:, c:c + P],
                                         mko_sb)
                # PV: one matmul per touched pv bank, batched over qbs.
                for g in (0, 1):
                    c0, c1 = max(lo, 4 * g), min(hi, 4 * g + 3)
                    if c0 > c1:
                        continue
                    if pv_t[g] is None:
                        pv_t[g] = pvp.tile([P, 512], f32, name="pvt",
                                           tag="pvt")
                    nc.tensor.matmul(
                        pv_t[g][0:VA, (c0 % 4) * P:(c1 % 4 + 1) * P],
                        lhsT=v_sb[:, kt, h, :],
                        rhs=p_t[:, (c0 - lo) * P:(c1 - lo + 1) * P],
                        start=(kt == 4 * g), stop=(kt == 4 * g + 7),
                        skip_group_check=True)
                for g in (0, 1):
                    if kt == 4 * g + 7:
                        # retire: normalize by the ones-row denominator.
                        # reciprocal runs lane-parallel via a DMA reshape;
                        # the [64,512] broadcast is a stride-0 DMA.  The
                        # vector-engine pieces (reciprocal, normalize-mul)
                        # are emitted a few steps late so the vector queue
                        # never parks on the DMA chain (it would block the
                        # mask-muls the PV matmuls need).
                        pv = pv_t[g]
                        rrow = epsp.tile([1, 512], f32, name="rrow",
                                         tag="rrow")
                        rcs = epsp.tile([128, 4], f32, name="rcs", tag="rcs")
                        rcr = epsp.tile([128, 4], f32, name="rcr", tag="rcr")
                        rc = epsp.tile([1, 512], f32, name="rc", tag="rc")
                        bc = bcp.tile([64, 512], f32, name="bc", tag="bc")
                        nc.scalar.copy(out=rrow, in_=pv[64:65, :])
                        nc.sync.dma_start(out=rcs, in_=rrow)
                        dst = ot_sb[hr:hr + 64, et, g * 512:(g + 1) * 512]
                        def mk1(rcs=rcs, rcr=rcr, rc=rc, bc=bc):
                            nc.vector.reciprocal(rcr, rcs)
                            nc.sync.dma_start(out=rc, in_=rcr)
                            nc.gpsimd.dma_start(out=bc,
                                                in_=_brd2(rc[0:1, :], 64))
                        def mk2(dst=dst, pv=pv, bc=bc):
                            nc.vector.tensor_mul(dst, pv[0:64, :], bc)
                        deferred.append((step[0] + 1, mk1))
                        deferred.append((step[0] + 3, mk2))
                step[0] += 1
                for due, fn in [x for x in deferred]:
                    if due <= step[0]:
                        fn()
                        deferred.remove((due, fn))

            def emit_att(h):
                for kt in range(NKT):
                    emit_scores(h, kt)
                    if len(pend) > 1:
                        emit_post()

            def flush_pend():
                while pend:
                    emit_post()
                for due, fn in deferred:
                    fn()
                deferred.clear()

            # ---- projections for the first two etile pairs ----
            proj_rope_et(wk_sb, kt_sb, 0, 0)
            proj_rope_et(wq_sb, qt_sb, W, 0)
            proj_rope_et(wk_sb, kt_sb, 0, 1)
            proj_rope_et(wq_sb, qt_sb, W, 1)

            # ---- V projection (scalar/vector engines are busy with
            # rope+exp meanwhile) ----
            for st_i in range(NKT):
                for eh in range(2):
                    ps = projp.tile([P, 512], f32, name="psv", tag="ps")
                    for d in range(8):
                        nc.tensor.matmul(
                            ps,
                            lhsT=xt_sb[:, d, st_i * P:(st_i + 1) * P],
                            rhs=wv_sb[:, d, eh * 512:(eh + 1) * 512],
                            start=(d == 0), stop=(d == 7))
                    # scatter heads into V_aug slots [st, h, 0:64]
                    nc.scalar.copy(
                        out=v_sb[:, st_i, eh * 8:(eh + 1) * 8, 0:64],
                        in_=ps[:, :].rearrange("p (h e) -> p h e", h=8))

            for et in range(8):
                emit_att(2 * et)
                emit_att(2 * et + 1)
                if et + 2 < 8:
                    proj_rope_et(wk_sb, kt_sb, 0, et + 2,
                                 mid_hook=flush_pend)
                    proj_rope_et(wq_sb, qt_sb, W, et + 2)

            # ---- output projection, out-tiles DMA'd as they finish ----
            first = [True]
            for qt_i in range(NQB):
                for nh in range(2):
                    ps = pvp.tile([P, 512], f32, name="pso", tag="pvt")
                    for p in range(8):
                        nc.tensor.matmul(
                            ps,
                            lhsT=ot_sb[:, p, qt_i * P:(qt_i + 1) * P],
                            rhs=wo_sb[:, p, nh * 512:(nh + 1) * 512],
                            start=(p == 0), stop=(p == 7))
                    if first[0]:
                        flush_pend()
                        first[0] = False
                    o_sb = osbp.tile([P, 512], f32, name="o_sb")
                    if nh == 0:
                        nc.vector.tensor_copy(o_sb, ps)
                    else:
                        nc.scalar.copy(out=o_sb, in_=ps)
                    (nc.sync if nh == 0 else nc.gpsimd).dma_start(
                        out=out[qt_i * P:(qt_i + 1) * P,
                                nh * 512:(nh + 1) * 512],
                        in_=o_sb)


def _prep_inputs(x, Wq, Wk, Wv, Wo):
    """Host-side shard/layout prep -> list of 8 per-core input dicts."""
    x2 = np.ascontiguousarray(x.reshape(S, DIM).astype(np.float32))
    # head-row permutation: [E0-15 | O0-15 | E16-31 | O16-31] so the rope
    # pair swap is a within-32-partition stream shuffle
    sigma = np.zeros(DIM, dtype=np.int64)
    j16 = np.arange(16)
    for h in range(H):
        b = h * 64
        sigma[b + j16] = b + 2 * j16              # E pairs 0-15
        sigma[b + 16 + j16] = b + 2 * j16 + 1     # O pairs 0-15
        sigma[b + 32 + j16] = b + 2 * (j16 + 16)  # E pairs 16-31
        sigma[b + 48 + j16] = b + 2 * (j16 + 16) + 1
    wq_h = np.ascontiguousarray(Wq.T[:, sigma]).astype(BF)
    wk_h = np.ascontiguousarray(Wk.T[:, sigma]).astype(BF)
    wv_h = np.ascontiguousarray(Wv.T).astype(BF)
    wo_h = np.ascontiguousarray(Wo.T).astype(BF)

    # rope tables in sigma row order, sin sign-folded:
    # row r (within 64-row head block): freq f(r), sign -1 on E rows
    rf = np.zeros(64, dtype=np.int64)
    sg = np.zeros(64, dtype=np.float32)
    rf[0:16], sg[0:16] = j16, -1.0          # E pairs 0-15
    rf[16:32], sg[16:32] = j16, 1.0         # O pairs 0-15
    rf[32:48], sg[32:48] = j16 + 16, -1.0   # E pairs 16-31
    rf[48:64], sg[48:64] = j16 + 16, 1.0    # O pairs 16-31
    rf = np.tile(rf, 2)
    sg = np.tile(sg, 2)

    kk = np.arange(P)[:, None]
    qq = np.arange(P)[None, :]
    mko_h = (kk > qq).astype(np.float32).astype(BF)    # keep k > q (oldest)
    mkd_h = (kk <= qq).astype(np.float32).astype(BF)   # keep k <= q (diag)

    inv_freq = 1.0 / (10000.0 ** (np.arange(0, D, 2, dtype=np.float32) / D))
    xT = x2.T  # [DIM, S]

    in_maps = []
    for core in range(NCORES):
        lo = core * SL - W
        xsh = np.zeros((DIM, SK), dtype=np.float32)
        if lo < 0:
            xsh[:, W:] = xT[:, :SL]
        else:
            xsh[:, :] = xT[:, lo:lo + SK]
        pos = np.arange(lo, lo + SK, dtype=np.float32)
        ang = pos[None, :] * inv_freq[rf][:, None]      # [128, SK]
        in_maps.append({
            "xt": xsh.astype(BF),
            "wq": wq_h, "wk": wk_h, "wv": wv_h, "wo": wo_h,
            "ropc": np.ascontiguousarray(np.cos(ang)).astype(BF),
            "rops": np.ascontiguousarray(
                sg[:, None] * np.sin(ang)).astype(BF),
            "vone": np.ascontiguousarray(
                (pos.reshape(NKT, P).T >= 0).astype(np.float32)).astype(BF),
            "mko": mko_h, "mkd": mkd_h,
        })
    return in_maps


def kernel(x, Wq, Wk, Wv, Wo, window_size, _trace=False, _trace_kwargs=None):
    assert int(window_size) == W
    if "nc" not in _compiled:
        _compiled["nc"] = _build()
    nc = _compiled["nc"]
    in_maps = _prep_inputs(np.asarray(x), np.asarray(Wq), np.asarray(Wk),
                           np.asarray(Wv), np.asarray(Wo))
    res = run_bass_kernel_spmd(nc, in_maps, core_ids=list(range(NCORES)),
                               trace=_trace, **(_trace_kwargs or {}))
    outp = np.concatenate([res.results[c]["out"] for c in range(NCORES)],
                          axis=0)
    _compiled["last_result"] = res
    return outp.reshape(1, S, DIM).astype(np.float32)


if __name__ == "__main__":
    np.random.seed(0)
    x = np.random.randn(1, S, DIM).astype(np.float32)
    sd = 1.0 / np.sqrt(DIM)
    ws = [np.random.randn(DIM, DIM).astype(np.float32) * sd for _ in range(4)]
    y = kernel(x, *ws, window_size=W)
    print("kernel output", y.shape, y.dtype, np.abs(y).max())


# revision 25
# speedup vs baseline: 1.0298x; 1.0298x over previous
"""Self-contained Trainium2 Bass kernel for sliding-window attention.

Problem (hardcoded): B=1, S=8192, dim=1024, H=16 heads, D=64 head dim,
window=512, fp32 I/O.  y = (softmax(mask(rope(xWq^T) rope(xWk^T)^T / 8)) xWv^T) Wo^T

Strategy: sequence-parallel over 8 NeuronCores. Each core owns 1024 query
rows and additionally recomputes K/V for the 512-row halo to its left
(core 0's halo is zero-padded and neutralized via a per-core "vones"
column so no collective is needed).  All matmuls run in bf16 (fp32 PSUM
accumulation).

v2 schedule changes vs baseline:
  - priority-ordered fine-grained input DMAs round-robined over 4 queues
    (first-needed tiles land first) + PE warm-up matmuls during the load
    to climb the tensor-engine p-state ramp.
  - rope uses a 16-wide even/odd interleave per head so the pair swap is
    a single DVE stream_shuffle (within-32-partition), plus a signed sin
    table: 1 shuffle + 2 muls + 1 add, all 128-partition-wide.
  - sliding-window causality applied as 0/1 mask multiplies on the DVE
    (post-exp) instead of rank-128 mask matmuls on the PE.
  - per-(head,kt) software pipelining: scores(kt+1) is emitted before
    exp/PV(kt) so the PE streams while the scalar engine runs exp.
  - softmax denominator broadcast via a stride-0 DMA instead of the slow
    gpsimd partition_broadcast.
  - output projection evictions alternate vector/scalar and out-tiles DMA
    immediately on rotating queues (no serial tail).

Layouts (per core):
  xT    [1024(d), 1536(s)]  x^T shard incl. halo (bf16)
  wq/wk [1024(d), 1024(e')] Wq^T / Wk^T with a per-head column permutation
                            (rope de-interleave: head h's rows are
                            [E0-15 | O0-15 | E16-31 | O16-31])
  wv    [1024(d), 1024(e)]  Wv^T (no permutation), wo = Wo^T
  Q^T/K^T are produced in [e', s] layout (weight-stationary matmuls) so
  attention needs no transposes: scores are computed transposed,
  S^T[k, q], the softmax denominator comes free from a ones-column
  appended to V, and PV directly yields o^T[e, q] — the lhsT of the
  output projection.
"""
import sys

sys.path.insert(0, "/opt/trn_rl_repo")

import numpy as np
import ml_dtypes

import concourse.bass as bass
import concourse.mybir as mybir
from concourse import bacc
from concourse.tile import TileContext
from concourse.bass_utils import run_bass_kernel_spmd

BF = ml_dtypes.bfloat16
NCORES = 8
S, DIM, H, D, W = 8192, 1024, 16, 64, 512
SL = S // NCORES          # 1024 own rows / core
SK = SL + W               # 1536 rows incl. left halo
P = 128
NKT = SK // P             # 12 kv tiles
NQB = SL // P             # 8 query tiles
dt = mybir.dt

_compiled = {}

SWAP16 = list(range(16, 32)) + list(range(16))  # within-32 E<->O block swap
VA = 80   # V_aug columns: 64 V + 1 ones + pad (32B-aligned stride)


def _build():
    nc = bacc.Bacc("TRN2", target_bir_lowering=False, debug=False,
                   num_devices=NCORES)
    def param(name, shape, dtype=dt.bfloat16, out=False):
        return nc.declare_dram_parameter(name, shape, dtype, isOutput=out)

    xt = param("xt", [DIM, SK])
    wq = param("wq", [DIM, DIM])
    wk = param("wk", [DIM, DIM])
    wv = param("wv", [DIM, DIM])
    wo = param("wo", [DIM, DIM])
    ropc = param("ropc", [P, SK])
    rops = param("rops", [P, SK])       # sign-folded sin table
    vone = param("vone", [P, NKT])      # host-expanded per-kt validity
    mko = param("mko", [P, P])          # keep-mask, oldest block  [k, q]
    mkd = param("mkd", [P, P])          # keep-mask, diagonal block [k, q]
    out = param("out", [SL, DIM], dt.float32, out=True)

    with TileContext(nc) as tc:
        _body(nc, tc, xt, wq, wk, wv, wo, ropc, rops, vone, mko, mkd, out)
    nc.compile()
    return nc


def _brd2(ap_slice, n):
    """Insert a stride-0 middle free dim of size n into a [p, c] AP."""
    return bass.AP(tensor=ap_slice.tensor, offset=ap_slice.offset,
                   ap=[ap_slice.ap[0], [0, n], ap_slice.ap[1]])


def _body(nc, tc, xt, wq, wk, wv, wo, ropc, rops, vone, mko, mkd, out):
    f32, bf16 = dt.float32, dt.bfloat16

    with tc.tile_pool(name="persist", bufs=1) as per:
        # long-lived SBUF tensors
        v_sb = per.tile([P, NKT, H, VA], bf16)    # V_aug: [V(64)|ones|pad]
        qt_sb = per.tile([P, 8, SL], bf16)        # Q^T (rope'd, sigma layout)
        kt_sb = per.tile([P, 8, SK], bf16)        # K^T
        ot_sb = per.tile([P, 8, SL], bf16)        # o^T (normalized)
        ropc_sb = per.tile([P, SK], bf16)
        rops_sb = per.tile([P, SK], bf16)
        mko_sb = per.tile([P, P], bf16)
        mkd_sb = per.tile([P, P], bf16)
        vone_sb = per.tile([P, NKT], bf16)

        with tc.tile_pool(name="xtp", bufs=1) as xtp, \
             tc.tile_pool(name="proj", bufs=2, space="PSUM") as projp, \
             tc.tile_pool(name="raw", bufs=3) as rawp, \
             tc.tile_pool(name="swp", bufs=3) as swp, \
             tc.tile_pool(name="mcp", bufs=2) as mcp, \
             tc.tile_pool(name="msp", bufs=2) as msp, \
             tc.tile_pool(name="wqk", bufs=1) as wqkp, \
             tc.tile_pool(name="pt", bufs=3) as ptp, \
             tc.tile_pool(name="st", bufs=3, space="PSUM") as stp, \
             tc.tile_pool(name="st5", bufs=1, space="PSUM") as st5p, \
             tc.tile_pool(name="pv", bufs=2, space="PSUM") as pvp, \
             tc.tile_pool(name="osb", bufs=2) as osbp, \
             tc.tile_pool(name="eps", bufs=1) as epsp, \
             tc.tile_pool(name="bcp", bufs=1) as bcp:
            xt_sb = xtp.tile([P, 8, SK], bf16)
            wk_sb = wqkp.tile([P, 8, DIM], bf16)
            wq_sb = wqkp.tile([P, 8, DIM], bf16)
            wv_sb = wqkp.tile([P, 8, DIM], bf16)
            wo_sb = wqkp.tile([P, 8, DIM], bf16)

            xt_r = xt.ap().rearrange("(d p) s -> p d s", p=P)
            wk_r = wk.ap().rearrange("(d p) e -> p d e", p=P)
            wq_r = wq.ap().rearrange("(d p) e -> p d e", p=P)
            wv_r = wv.ap().rearrange("(d p) e -> p d e", p=P)
            wo_r = wo.ap().rearrange("(e p) n -> p e n", p=P)

            # ---- priority-ordered input DMAs, round-robin over 3 queues
            # (each queue drains in order, so each queue's prefix is the
            # first-needed data; ~0.65us issue cost per DMA dominates below
            # ~256KB, so pieces are kept large) ----
            dmas = [
                (ropc_sb[:, :], ropc[:, :]),
                (rops_sb[:, :], rops[:, :]),
                (mko_sb[:, :], mko[:, :]),
                (mkd_sb[:, :], mkd[:, :]),
                (vone_sb[:, :], vone[:, :]),
            ]
            for d in range(8):
                dmas.append((xt_sb[:, d, 0:768], xt_r[:, d, 0:768]))
            for d in range(8):
                dmas.append((wk_sb[:, d, :], wk_r[:, d, :]))
            for d in range(8):
                dmas.append((xt_sb[:, d, 768:1536], xt_r[:, d, 768:1536]))
            for d in range(8):
                dmas.append((wq_sb[:, d, :], wq_r[:, d, :]))
            for d in range(8):
                dmas.append((wv_sb[:, d, :], wv_r[:, d, :]))
            for d in range(8):
                dmas.append((wo_sb[:, d, :], wo_r[:, d, :]))
            # first waves may use the scalar queue (it is idle until the
            # first projection eviction ~12us in); later waves must not sit
            # ahead of evictions/exps on the scalar queue
            DQ3 = [nc.sync, nc.scalar, nc.gpsimd]
            DQ2 = [nc.sync, nc.gpsimd]
            for i, (o_ap, i_ap) in enumerate(dmas):
                if i < 30:
                    DQ3[i % 3].dma_start(out=o_ap, in_=i_ap)
                else:
                    DQ2[i % 2].dma_start(out=o_ap, in_=i_ap)

            # ---- PE warm-up: dummy matmuls on the rope table while the
            # real inputs stream in, to climb the p-state ramp ----
            warm_ps = projp.tile([P, 512], f32, name="warm", tag="ps")
            for _ in range(8):
                nc.tensor.matmul(warm_ps, lhsT=ropc_sb[:, 0:P],
                                 rhs=ropc_sb[:, 0:512], start=True, stop=True)

            # ones column of V_aug (per-core halo-validity mask)
            for kt_i in range(NKT):
                nc.vector.tensor_copy(
                    out=v_sb[:, kt_i, :, 64],
                    in_=_brd2(vone_sb[:, kt_i:kt_i + 1], H))

            def rope_chunk(raw, dst, c0):
                # raw [P, 512] bf16 -> dst [P, 512] (rope'd).
                # dst = raw*cos + swap16(raw)*sin_signed
                cseg = slice(c0, c0 + 512)
                rsw = swp.tile([P, 512], bf16, name="rsw", tag="rsw")
                mcos = mcp.tile([P, 512], bf16, name="mcos", tag="mcos")
                msw = msp.tile([P, 512], bf16, name="msw", tag="msw")
                nc.vector.stream_shuffle(rsw, raw, SWAP16)
                nc.vector.tensor_mul(mcos, raw, ropc_sb[:, cseg])
                nc.vector.tensor_mul(msw, rsw, rops_sb[:, cseg])
                nc.vector.tensor_add(dst, mcos, msw)

            def proj_rope_et(w_sb, dst, s0, et, mid_hook=None):
                # dst[:, et, s0:SK] = rope((W^T)^T @ xT[:, s0:SK])
                for si, sp in enumerate(range(s0, SK, 512)):
                    ps = projp.tile([P, 512], f32, name="ps", tag="ps")
                    for d in range(8):
                        nc.tensor.matmul(
                            ps,
                            lhsT=w_sb[:, d, et * P:(et + 1) * P],
                            rhs=xt_sb[:, d, sp:sp + 512],
                            start=(d == 0), stop=(d == 7))
                    if si == 0 and mid_hook is not None:
                        mid_hook()
                    raw = rawp.tile([P, 512], bf16, name="raw", tag="raw")
                    nc.scalar.copy(out=raw, in_=ps)
                    rope_chunk(raw, dst[:, et, sp - s0:sp - s0 + 512], sp)

            # ---- attention with cross-(head,kt) software pipelining ----
            pv_state = {}     # h -> [pv_t_g0, pv_t_g1]
            pend = []         # [(h, kt, st_ps, lo, hi)]
            step = [0]        # global post counter
            deferred = []     # [(due_step, fn)] late-emitted retire tails

            def emit_scores(h, kt):
                et, hr = h // 2, (h % 2) * 64
                lo, hi = max(kt - 4, 0), min(kt, 7)
                nqb = hi - lo + 1
                n0 = min(nqb, 4) * P
                kh = kt_sb[hr:hr + 64, et, kt * P:(kt + 1) * P]
                st_ps = stp.tile([P, 512], f32, name="st_ps")
                st5 = None
                nc.tensor.matmul(
                    st_ps[:, 0:n0], lhsT=kh,
                    rhs=qt_sb[hr:hr + 64, et, lo * P:lo * P + n0],
                    start=True, stop=True)
                if nqb == 5:
                    st5 = st5p.tile([P, P], f32, name="st5")
                    nc.tensor.matmul(
                        st5, lhsT=kh,
                        rhs=qt_sb[hr:hr + 64, et, (lo + 4) * P:(lo + 5) * P],
                        start=True, stop=True)
                pend.append((h, kt, st_ps, st5, lo, hi))

            def emit_post():
                if not pend:
                    return
                h, kt, st_ps, st5, lo, hi = pend.pop(0)
                et, hr = h // 2, (h % 2) * 64
                nqb = hi - lo + 1
                n0 = min(nqb, 4) * P
                pv_t = pv_state.setdefault(h, [None, None])
                p_t = ptp.tile([P, 640], bf16, name="p_t")
                nc.scalar.activation(
                    out=p_t[:, 0:n0], in_=st_ps[:, 0:n0],
                    func=mybir.ActivationFunctionType.Exp, scale=0.125)
                if st5 is not None:
                    nc.scalar.activation(
                        out=p_t[:, 512:640], in_=st5,
                        func=mybir.ActivationFunctionType.Exp, scale=0.125)
                if kt >= 4:          # causal (diagonal) block sits at col 0
                    nc.vector.tensor_mul(p_t[:, 0:P], p_t[:, 0:P], mkd_sb)
                if kt <= 7:          # oldest block sits at col kt-lo
                    c = (kt - lo) * P
                    nc.vector.tensor_mul(p_t[:, c:c + P], p_t[:, c:c + P],
                                         mko_sb)
                # PV: one matmul per touched pv bank, batched over qbs.
                for g in (0, 1):
                    c0, c1 = max(lo, 4 * g), min(hi, 4 * g + 3)
                    if c0 > c1:
                        continue
                    if pv_t[g] is None:
                        pv_t[g] = pvp.tile([P, 512], f32, name="pvt",
                                           tag="pvt")
                    nc.tensor.matmul(
                        pv_t[g][0:VA, (c0 % 4) * P:(c1 % 4 + 1) * P],
                        lhsT=v_sb[:, kt, h, :],
                        rhs=p_t[:, (c0 - lo) * P:(c1 - lo + 1) * P],
                        start=(kt == 4 * g), stop=(kt == 4 * g + 7),
                        skip_group_check=True)
                for g in (0, 1):
                    if kt == 4 * g + 7:
                        # retire: normalize by the ones-row denominator.
                        # reciprocal runs lane-parallel via a DMA reshape;
                        # the [64,512] broadcast is a stride-0 DMA.  Every
                        # hop is emitted one pipeline step after its
                        # producer so no queue ever parks on the chain
                        # (a parked vector queue would block the mask-muls
                        # the PV matmuls need).
                        pv = pv_t[g]
                        rrow = epsp.tile([1, 512], f32, name="rrow",
                                         tag="rrow")
                        rcs = epsp.tile([128, 4], f32, name="rcs", tag="rcs")
                        rcr = epsp.tile([128, 4], f32, name="rcr", tag="rcr")
                        rc = epsp.tile([1, 512], f32, name="rc", tag="rc")
                        bc = bcp.tile([64, 512], f32, name="bc", tag="bc")
                        nc.scalar.copy(out=rrow, in_=pv[64:65, :])
                        dst = ot_sb[hr:hr + 64, et, g * 512:(g + 1) * 512]
                        s0 = step[0]
                        deferred.append((s0 + 1, lambda rcs=rcs, rrow=rrow:
                                         nc.sync.dma_start(out=rcs,
                                                           in_=rrow)))
                        def mk2(rcr=rcr, rcs=rcs, rc=rc):
                            nc.vector.reciprocal(rcr, rcs)
                            nc.sync.dma_start(out=rc, in_=rcr)
                        deferred.append((s0 + 2, mk2))
                        deferred.append((s0 + 3, lambda bc=bc, rc=rc:
                                         nc.gpsimd.dma_start(
                                             out=bc,
                                             in_=_brd2(rc[0:1, :], 64))))
                        deferred.append((s0 + 4, lambda dst=dst, pv=pv,
                                         bc=bc:
                                         nc.vector.tensor_mul(
                                             dst, pv[0:64, :], bc)))
                step[0] += 1
                for due, fn in [x for x in deferred]:
                    if due <= step[0]:
                        fn()
                        deferred.remove((due, fn))

            def emit_att(h):
                for kt in range(NKT):
                    emit_scores(h, kt)
                    if len(pend) > 2:
                        emit_post()

            def flush_pend():
                while pend:
                    emit_post()
                for due, fn in deferred:
                    fn()
                deferred.clear()

            # ---- projections for the first two etile pairs ----
            proj_rope_et(wk_sb, kt_sb, 0, 0)
            proj_rope_et(wq_sb, qt_sb, W, 0)
            proj_rope_et(wk_sb, kt_sb, 0, 1)
            proj_rope_et(wq_sb, qt_sb, W, 1)

            # ---- V projection (scalar/vector engines are busy with
            # rope+exp meanwhile) ----
            for st_i in range(NKT):
                for eh in range(2):
                    ps = projp.tile([P, 512], f32, name="psv", tag="ps")
                    for d in range(8):
                        nc.tensor.matmul(
                            ps,
                            lhsT=xt_sb[:, d, st_i * P:(st_i + 1) * P],
                            rhs=wv_sb[:, d, eh * 512:(eh + 1) * 512],
                            start=(d == 0), stop=(d == 7))
                    # scatter heads into V_aug slots [st, h, 0:64]
                    nc.scalar.copy(
                        out=v_sb[:, st_i, eh * 8:(eh + 1) * 8, 0:64],
                        in_=ps[:, :].rearrange("p (h e) -> p h e", h=8))

            for et in range(8):
                emit_att(2 * et)
                emit_att(2 * et + 1)
                if et + 2 < 8:
                    proj_rope_et(wk_sb, kt_sb, 0, et + 2,
                                 mid_hook=flush_pend)
                    proj_rope_et(wq_sb, qt_sb, W, et + 2)

            # ---- output projection, out-tiles DMA'd as they finish ----
            flush_pend()
            for qt_i in range(NQB):
                for nh in range(2):
                    ps = pvp.tile([P, 512], f32, name="pso", tag="pvt")
                    for p in range(8):
                        nc.tensor.matmul(
                            ps,
                            lhsT=ot_sb[:, p, qt_i * P:(qt_i + 1) * P],
                            rhs=wo_sb[:, p, nh * 512:(nh + 1) * 512],
                            start=(p == 0), stop=(p == 7))
                    o_sb = osbp.tile([P, 512], f32, name="o_sb")
                    if nh == 0:
                        nc.vector.tensor_copy(o_sb, ps)
                    else:
                        nc.scalar.copy(out=o_sb, in_=ps)
                    (nc.sync if nh == 0 else nc.gpsimd).dma_start(
                        out=out[qt_i * P:(qt_i + 1) * P,
                                nh * 512:(nh + 1) * 512],
                        in_=o_sb)


def _prep_inputs(x, Wq, Wk, Wv, Wo):
    """Host-side shard/layout prep -> list of 8 per-core input dicts."""
    x2 = np.ascontiguousarray(x.reshape(S, DIM).astype(np.float32))
    # head-row permutation: [E0-15 | O0-15 | E16-31 | O16-31] so the rope
    # pair swap is a within-32-partition stream shuffle
    sigma = np.zeros(DIM, dtype=np.int64)
    j16 = np.arange(16)
    for h in range(H):
        b = h * 64
        sigma[b + j16] = b + 2 * j16              # E pairs 0-15
        sigma[b + 16 + j16] = b + 2 * j16 + 1     # O pairs 0-15
        sigma[b + 32 + j16] = b + 2 * (j16 + 16)  # E pairs 16-31
        sigma[b + 48 + j16] = b + 2 * (j16 + 16) + 1
    wq_h = np.ascontiguousarray(Wq.T[:, sigma]).astype(BF)
    wk_h = np.ascontiguousarray(Wk.T[:, sigma]).astype(BF)
    wv_h = np.ascontiguousarray(Wv.T).astype(BF)
    wo_h = np.ascontiguousarray(Wo.T).astype(BF)

    # rope tables in sigma row order, sin sign-folded:
    # row r (within 64-row head block): freq f(r), sign -1 on E rows
    rf = np.zeros(64, dtype=np.int64)
    sg = np.zeros(64, dtype=np.float32)
    rf[0:16], sg[0:16] = j16, -1.0          # E pairs 0-15
    rf[16:32], sg[16:32] = j16, 1.0         # O pairs 0-15
    rf[32:48], sg[32:48] = j16 + 16, -1.0   # E pairs 16-31
    rf[48:64], sg[48:64] = j16 + 16, 1.0    # O pairs 16-31
    rf = np.tile(rf, 2)
    sg = np.tile(sg, 2)

    kk = np.arange(P)[:, None]
    qq = np.arange(P)[None, :]
    mko_h = (kk > qq).astype(np.float32).astype(BF)    # keep k > q (oldest)
    mkd_h = (kk <= qq).astype(np.float32).astype(BF)   # keep k <= q (diag)

    inv_freq = 1.0 / (10000.0 ** (np.arange(0, D, 2, dtype=np.float32) / D))
    xT = x2.T  # [DIM, S]

    in_maps = []
    for core in range(NCORES):
        lo = core * SL - W
        xsh = np.zeros((DIM, SK), dtype=np.float32)
        if lo < 0:
            xsh[:, W:] = xT[:, :SL]
        else:
            xsh[:, :] = xT[:, lo:lo + SK]
        pos = np.arange(lo, lo + SK, dtype=np.float32)
        ang = pos[None, :] * inv_freq[rf][:, None]      # [128, SK]
        in_maps.append({
            "xt": xsh.astype(BF),
            "wq": wq_h, "wk": wk_h, "wv": wv_h, "wo": wo_h,
            "ropc": np.ascontiguousarray(np.cos(ang)).astype(BF),
            "rops": np.ascontiguousarray(
                sg[:, None] * np.sin(ang)).astype(BF),
            "vone": np.ascontiguousarray(
                (pos.reshape(NKT, P).T >= 0).astype(np.float32)).astype(BF),
            "mko": mko_h, "mkd": mkd_h,
        })
    return in_maps


def kernel(x, Wq, Wk, Wv, Wo, window_size, _trace=False, _trace_kwargs=None):
    assert int(window_size) == W
    if "nc" not in _compiled:
        _compiled["nc"] = _build()
    nc = _compiled["nc"]
    in_maps = _prep_inputs(np.asarray(x), np.asarray(Wq), np.asarray(Wk),
                           np.asarray(Wv), np.asarray(Wo))
    res = run_bass_kernel_spmd(nc, in_maps, core_ids=list(range(NCORES)),
                               trace=_trace, **(_trace_kwargs or {}))
    outp = np.concatenate([res.results[c]["out"] for c in range(NCORES)],
                          axis=0)
    _compiled["last_result"] = res
    return outp.reshape(1, S, DIM).astype(np.float32)


if __name__ == "__main__":
    np.random.seed(0)
    x = np.random.randn(1, S, DIM).astype(np.float32)
    sd = 1.0 / np.sqrt(DIM)
    ws = [np.random.randn(DIM, DIM).astype(np.float32) * sd for _ in range(4)]
    y = kernel(x, *ws, window_size=W)
    print("kernel output", y.shape, y.dtype, np.abs(y).max())


# revision 28
# speedup vs baseline: 1.0681x; 1.0372x over previous
"""Self-contained Trainium2 Bass kernel for sliding-window attention.

Problem (hardcoded): B=1, S=8192, dim=1024, H=16 heads, D=64 head dim,
window=512, fp32 I/O.  y = (softmax(mask(rope(xWq^T) rope(xWk^T)^T / 8)) xWv^T) Wo^T

Strategy: sequence-parallel over 8 NeuronCores. Each core owns 1024 query
rows and additionally recomputes K/V for the 512-row halo to its left
(core 0's halo is zero-padded and neutralized via a per-core "vones"
column so no collective is needed).  All matmuls run in bf16 (fp32 PSUM
accumulation).

v2 schedule changes vs baseline:
  - priority-ordered fine-grained input DMAs round-robined over 4 queues
    (first-needed tiles land first) + PE warm-up matmuls during the load
    to climb the tensor-engine p-state ramp.
  - rope uses a 16-wide even/odd interleave per head so the pair swap is
    a single DVE stream_shuffle (within-32-partition), plus a signed sin
    table: 1 shuffle + 2 muls + 1 add, all 128-partition-wide.
  - sliding-window causality applied as 0/1 mask multiplies on the DVE
    (post-exp) instead of rank-128 mask matmuls on the PE.
  - per-(head,kt) software pipelining: scores(kt+1) is emitted before
    exp/PV(kt) so the PE streams while the scalar engine runs exp.
  - softmax denominator broadcast via a stride-0 DMA instead of the slow
    gpsimd partition_broadcast.
  - output projection evictions alternate vector/scalar and out-tiles DMA
    immediately on rotating queues (no serial tail).

Layouts (per core):
  xT    [1024(d), 1536(s)]  x^T shard incl. halo (bf16)
  wq/wk [1024(d), 1024(e')] Wq^T / Wk^T with a per-head column permutation
                            (rope de-interleave: head h's rows are
                            [E0-15 | O0-15 | E16-31 | O16-31])
  wv    [1024(d), 1024(e)]  Wv^T (no permutation), wo = Wo^T
  Q^T/K^T are produced in [e', s] layout (weight-stationary matmuls) so
  attention needs no transposes: scores are computed transposed,
  S^T[k, q], the softmax denominator comes free from a ones-column
  appended to V, and PV directly yields o^T[e, q] — the lhsT of the
  output projection.
"""
import sys

sys.path.insert(0, "/opt/trn_rl_repo")

import numpy as np
import ml_dtypes

import concourse.bass as bass
import concourse.mybir as mybir
from concourse import bacc
from concourse.tile import TileContext
from concourse.bass_utils import run_bass_kernel_spmd

BF = ml_dtypes.bfloat16
NCORES = 8
S, DIM, H, D, W = 8192, 1024, 16, 64, 512
SL = S // NCORES          # 1024 own rows / core
SK = SL + W               # 1536 rows incl. left halo
P = 128
NKT = SK // P             # 12 kv tiles
NQB = SL // P             # 8 query tiles
dt = mybir.dt

_compiled = {}

SWAP16 = list(range(16, 32)) + list(range(16))  # within-32 E<->O block swap
VA = 80   # V_aug columns: 64 V + 1 ones + pad (32B-aligned stride)


def _build():
    nc = bacc.Bacc("TRN2", target_bir_lowering=False, debug=False,
                   num_devices=NCORES)
    def param(name, shape, dtype=dt.bfloat16, out=False):
        return nc.declare_dram_parameter(name, shape, dtype, isOutput=out)

    xt = param("xt", [DIM, SK])
    wq = param("wq", [DIM, DIM])
    wk = param("wk", [DIM, DIM])
    wv = param("wv", [DIM, DIM])
    wo = param("wo", [DIM, DIM])
    ropc = param("ropc", [P, SK])
    rops = param("rops", [P, SK])       # sign-folded sin table
    vone = param("vone", [P, NKT])      # host-expanded per-kt validity
    mko = param("mko", [P, P])          # keep-mask, oldest block  [k, q]
    mkd = param("mkd", [P, P])          # keep-mask, diagonal block [k, q]
    out = param("out", [SL, DIM], dt.float32, out=True)

    with TileContext(nc) as tc:
        _body(nc, tc, xt, wq, wk, wv, wo, ropc, rops, vone, mko, mkd, out)
    nc.compile()
    return nc


def _brd2(ap_slice, n):
    """Insert a stride-0 middle free dim of size n into a [p, c] AP."""
    return bass.AP(tensor=ap_slice.tensor, offset=ap_slice.offset,
                   ap=[ap_slice.ap[0], [0, n], ap_slice.ap[1]])


def _body(nc, tc, xt, wq, wk, wv, wo, ropc, rops, vone, mko, mkd, out):
    f32, bf16 = dt.float32, dt.bfloat16

    with tc.tile_pool(name="persist", bufs=1) as per:
        # long-lived SBUF tensors
        v_sb = per.tile([P, NKT, H, VA], bf16)    # V_aug: [V(64)|ones|pad]
        qt_sb = per.tile([P, 8, SL], bf16)        # Q^T (rope'd, sigma layout)
        kt_sb = per.tile([P, 8, SK], bf16)        # K^T
        ot_sb = per.tile([P, 8, SL], bf16)        # o^T (normalized)
        ropc_sb = per.tile([P, SK], bf16)
        rops_sb = per.tile([P, SK], bf16)
        mko_sb = per.tile([P, P], bf16)
        mkd_sb = per.tile([P, P], bf16)
        vone_sb = per.tile([P, NKT], bf16)

        with tc.tile_pool(name="xtp", bufs=1) as xtp, \
             tc.tile_pool(name="proj", bufs=2, space="PSUM") as projp, \
             tc.tile_pool(name="raw", bufs=3) as rawp, \
             tc.tile_pool(name="swp", bufs=3) as swp, \
             tc.tile_pool(name="mcp", bufs=2) as mcp, \
             tc.tile_pool(name="msp", bufs=2) as msp, \
             tc.tile_pool(name="wqk", bufs=1) as wqkp, \
             tc.tile_pool(name="pt", bufs=3) as ptp, \
             tc.tile_pool(name="st", bufs=3, space="PSUM") as stp, \
             tc.tile_pool(name="st5", bufs=1, space="PSUM") as st5p, \
             tc.tile_pool(name="pv", bufs=2, space="PSUM") as pvp, \
             tc.tile_pool(name="osb", bufs=2) as osbp, \
             tc.tile_pool(name="eps", bufs=1) as epsp, \
             tc.tile_pool(name="bcp", bufs=1) as bcp:
            xt_sb = xtp.tile([P, 8, SK], bf16)
            wk_sb = wqkp.tile([P, 8, DIM], bf16)
            wq_sb = wqkp.tile([P, 8, DIM], bf16)
            wv_sb = wqkp.tile([P, 8, DIM], bf16)
            wo_sb = wqkp.tile([P, 8, DIM], bf16)

            xt_r = xt.ap().rearrange("(d p) s -> p d s", p=P)
            wk_r = wk.ap().rearrange("(d p) e -> p d e", p=P)
            wq_r = wq.ap().rearrange("(d p) e -> p d e", p=P)
            wv_r = wv.ap().rearrange("(d p) e -> p d e", p=P)
            wo_r = wo.ap().rearrange("(e p) n -> p e n", p=P)

            # ---- priority-ordered input DMAs, round-robin over 3 queues
            # (each queue drains in order, so each queue's prefix is the
            # first-needed data; ~0.65us issue cost per DMA dominates below
            # ~256KB, so pieces are kept large) ----
            dmas = [
                (ropc_sb[:, :], ropc[:, :]),
                (rops_sb[:, :], rops[:, :]),
                (mko_sb[:, :], mko[:, :]),
                (mkd_sb[:, :], mkd[:, :]),
                (vone_sb[:, :], vone[:, :]),
            ]
            for d in range(8):
                dmas.append((xt_sb[:, d, 0:768], xt_r[:, d, 0:768]))
            for d in range(8):
                dmas.append((wk_sb[:, d, :], wk_r[:, d, :]))
            for d in range(8):
                dmas.append((xt_sb[:, d, 768:1536], xt_r[:, d, 768:1536]))
            for d in range(8):
                dmas.append((wq_sb[:, d, :], wq_r[:, d, :]))
            for d in range(8):
                dmas.append((wv_sb[:, d, :], wv_r[:, d, :]))
            for d in range(8):
                dmas.append((wo_sb[:, d, :], wo_r[:, d, :]))
            # first waves may use the scalar queue (it is idle until the
            # first projection eviction ~12us in); later waves must not sit
            # ahead of evictions/exps on the scalar queue
            DQ3 = [nc.sync, nc.scalar, nc.gpsimd]
            DQ2 = [nc.sync, nc.gpsimd]
            for i, (o_ap, i_ap) in enumerate(dmas):
                if i < 30:
                    DQ3[i % 3].dma_start(out=o_ap, in_=i_ap)
                else:
                    DQ2[i % 2].dma_start(out=o_ap, in_=i_ap)

            # ---- PE warm-up: dummy matmuls on the rope table while the
            # real inputs stream in, to climb the p-state ramp ----
            warm_ps = projp.tile([P, 512], f32, name="warm", tag="ps")
            for _ in range(8):
                nc.tensor.matmul(warm_ps, lhsT=ropc_sb[:, 0:P],
                                 rhs=ropc_sb[:, 0:512], start=True, stop=True)

            # ones column of V_aug (per-core halo-validity mask)
            for kt_i in range(NKT):
                nc.vector.tensor_copy(
                    out=v_sb[:, kt_i, :, 64],
                    in_=_brd2(vone_sb[:, kt_i:kt_i + 1], H))

            def rope_chunk(raw, dst, c0):
                # raw [P, 512] bf16 -> dst [P, 512] (rope'd).
                # dst = raw*cos + swap16(raw)*sin_signed
                cseg = slice(c0, c0 + 512)
                rsw = swp.tile([P, 512], bf16, name="rsw", tag="rsw")
                mcos = mcp.tile([P, 512], bf16, name="mcos", tag="mcos")
                msw = msp.tile([P, 512], bf16, name="msw", tag="msw")
                nc.vector.stream_shuffle(rsw, raw, SWAP16)
                nc.vector.tensor_mul(mcos, raw, ropc_sb[:, cseg])
                nc.vector.tensor_mul(msw, rsw, rops_sb[:, cseg])
                nc.vector.tensor_add(dst, mcos, msw)

            def proj_rope_et(w_sb, dst, s0, et, mid_hook=None):
                # dst[:, et, s0:SK] = rope((W^T)^T @ xT[:, s0:SK])
                for si, sp in enumerate(range(s0, SK, 512)):
                    ps = projp.tile([P, 512], f32, name="ps", tag="ps")
                    for d in range(8):
                        nc.tensor.matmul(
                            ps,
                            lhsT=w_sb[:, d, et * P:(et + 1) * P],
                            rhs=xt_sb[:, d, sp:sp + 512],
                            start=(d == 0), stop=(d == 7))
                    if si == 0 and mid_hook is not None:
                        mid_hook()
                    raw = rawp.tile([P, 512], bf16, name="raw", tag="raw")
                    nc.scalar.copy(out=raw, in_=ps)
                    rope_chunk(raw, dst[:, et, sp - s0:sp - s0 + 512], sp)

            # ---- attention with cross-(head,kt) software pipelining ----
            pv_state = {}     # h -> [pv_t_g0, pv_t_g1]
            pend = []         # [(h, kt, st_ps, lo, hi)]
            step = [0]        # global post counter
            deferred = []     # [(due_step, fn)] late-emitted retire tails

            def emit_scores(h, kt):
                et, hr = h // 2, (h % 2) * 64
                lo, hi = max(kt - 4, 0), min(kt, 7)
                nqb = hi - lo + 1
                n0 = min(nqb, 4) * P
                kh = kt_sb[hr:hr + 64, et, kt * P:(kt + 1) * P]
                st_ps = stp.tile([P, 512], f32, name="st_ps")
                st5 = None
                nc.tensor.matmul(
                    st_ps[:, 0:n0], lhsT=kh,
                    rhs=qt_sb[hr:hr + 64, et, lo * P:lo * P + n0],
                    start=True, stop=True)
                if nqb == 5:
                    st5 = st5p.tile([P, P], f32, name="st5")
                    nc.tensor.matmul(
                        st5, lhsT=kh,
                        rhs=qt_sb[hr:hr + 64, et, (lo + 4) * P:(lo + 5) * P],
                        start=True, stop=True)
                pend.append((h, kt, st_ps, st5, lo, hi))

            def emit_post():
                if not pend:
                    return
                h, kt, st_ps, st5, lo, hi = pend.pop(0)
                et, hr = h // 2, (h % 2) * 64
                nqb = hi - lo + 1
                n0 = min(nqb, 4) * P
                pv_t = pv_state.setdefault(h, [None, None])
                p_t = ptp.tile([P, 640], bf16, name="p_t")
                nc.scalar.activation(
                    out=p_t[:, 0:n0], in_=st_ps[:, 0:n0],
                    func=mybir.ActivationFunctionType.Exp, scale=0.125)
                if st5 is not None:
                    nc.scalar.activation(
                        out=p_t[:, 512:640], in_=st5,
                        func=mybir.ActivationFunctionType.Exp, scale=0.125)
                # mask-muls run on the (otherwise idle) Pool engine so the
                # PV critical path never sits behind vector-queue parks
                if kt >= 4:          # causal (diagonal) block sits at col 0
                    nc.gpsimd.tensor_mul(p_t[:, 0:P], p_t[:, 0:P], mkd_sb)
                if kt <= 7:          # oldest block sits at col kt-lo
                    c = (kt - lo) * P
                    nc.gpsimd.tensor_mul(p_t[:, c:c + P], p_t[:, c:c + P],
                                         mko_sb)
                # PV: one matmul per touched pv bank, batched over qbs.
                for g in (0, 1):
                    c0, c1 = max(lo, 4 * g), min(hi, 4 * g + 3)
                    if c0 > c1:
                        continue
                    if pv_t[g] is None:
                        pv_t[g] = pvp.tile([P, 512], f32, name="pvt",
                                           tag="pvt")
                    nc.tensor.matmul(
                        pv_t[g][0:VA, (c0 % 4) * P:(c1 % 4 + 1) * P],
                        lhsT=v_sb[:, kt, h, :],
                        rhs=p_t[:, (c0 - lo) * P:(c1 - lo + 1) * P],
                        start=(kt == 4 * g), stop=(kt == 4 * g + 7),
                        skip_group_check=True)
                for g in (0, 1):
                    if kt == 4 * g + 7:
                        # retire: normalize by the ones-row denominator.
                        # reciprocal runs lane-parallel via a DMA reshape;
                        # the [64,512] broadcast is a stride-0 DMA.  Every
                        # hop is emitted one pipeline step after its
                        # producer so no queue ever parks on the chain
                        # (a parked vector queue would block the mask-muls
                        # the PV matmuls need).
                        pv = pv_t[g]
                        rrow = epsp.tile([1, 512], f32, name="rrow",
                                         tag="rrow")
                        rcs = epsp.tile([128, 4], f32, name="rcs", tag="rcs")
                        rcr = epsp.tile([128, 4], f32, name="rcr", tag="rcr")
                        rc = epsp.tile([1, 512], f32, name="rc", tag="rc")
                        bc = bcp.tile([64, 512], f32, name="bc", tag="bc")
                        nc.scalar.copy(out=rrow, in_=pv[64:65, :])
                        dst = ot_sb[hr:hr + 64, et, g * 512:(g + 1) * 512]
                        s0 = step[0]
                        deferred.append((s0 + 1, lambda rcs=rcs, rrow=rrow:
                                         nc.sync.dma_start(out=rcs,
                                                           in_=rrow)))
                        def mk2(rcr=rcr, rcs=rcs, rc=rc):
                            nc.vector.reciprocal(rcr, rcs)
                            nc.sync.dma_start(out=rc, in_=rcr)
                        deferred.append((s0 + 2, mk2))
                        deferred.append((s0 + 3, lambda bc=bc, rc=rc:
                                         nc.sync.dma_start(
                                             out=bc,
                                             in_=_brd2(rc[0:1, :], 64))))
                        deferred.append((s0 + 4, lambda dst=dst, pv=pv,
                                         bc=bc:
                                         nc.vector.tensor_mul(
                                             dst, pv[0:64, :], bc)))
                step[0] += 1
                for due, fn in [x for x in deferred]:
                    if due <= step[0]:
                        fn()
                        deferred.remove((due, fn))

            def emit_att(h):
                for kt in range(NKT):
                    emit_scores(h, kt)
                    if len(pend) > 2:
                        emit_post()

            def flush_pend():
                while pend:
                    emit_post()
                for due, fn in deferred:
                    fn()
                deferred.clear()

            # ---- projections for the first two etile pairs ----
            proj_rope_et(wk_sb, kt_sb, 0, 0)
            proj_rope_et(wq_sb, qt_sb, W, 0)
            proj_rope_et(wk_sb, kt_sb, 0, 1)
            proj_rope_et(wq_sb, qt_sb, W, 1)

            # ---- V projection (scalar/vector engines are busy with
            # rope+exp meanwhile) ----
            for st_i in range(NKT):
                for eh in range(2):
                    ps = projp.tile([P, 512], f32, name="psv", tag="ps")
                    for d in range(8):
                        nc.tensor.matmul(
                            ps,
                            lhsT=xt_sb[:, d, st_i * P:(st_i + 1) * P],
                            rhs=wv_sb[:, d, eh * 512:(eh + 1) * 512],
                            start=(d == 0), stop=(d == 7))
                    # scatter heads into V_aug slots [st, h, 0:64]
                    nc.scalar.copy(
                        out=v_sb[:, st_i, eh * 8:(eh + 1) * 8, 0:64],
                        in_=ps[:, :].rearrange("p (h e) -> p h e", h=8))

            for et in range(8):
                emit_att(2 * et)
                emit_att(2 * et + 1)
                if et + 2 < 8:
                    proj_rope_et(wk_sb, kt_sb, 0, et + 2,
                                 mid_hook=flush_pend)
                    proj_rope_et(wq_sb, qt_sb, W, et + 2)

            # ---- output projection, out-tiles DMA'd as they finish ----
            flush_pend()
            for qt_i in range(NQB):
                for nh in range(2):
                    ps = pvp.tile([P, 512], f32, name="pso", tag="pvt")
                    for p in range(8):
                        nc.tensor.matmul(
                            ps,
                            lhsT=ot_sb[:, p, qt_i * P:(qt_i + 1) * P],
                            rhs=wo_sb[:, p, nh * 512:(nh + 1) * 512],
                            start=(p == 0), stop=(p == 7))
                    o_sb = osbp.tile([P, 512], f32, name="o_sb")
                    if nh == 0:
                        nc.vector.tensor_copy(o_sb, ps)
                    else:
                        nc.scalar.copy(out=o_sb, in_=ps)
                    DQ3[(qt_i * 2 + nh) % 3].dma_start(
                        out=out[qt_i * P:(qt_i + 1) * P,
                                nh * 512:(nh + 1) * 512],
                        in_=o_sb)


def _prep_inputs(x, Wq, Wk, Wv, Wo):
    """Host-side shard/layout prep -> list of 8 per-core input dicts."""
    x2 = np.ascontiguousarray(x.reshape(S, DIM).astype(np.float32))
    # head-row permutation: [E0-15 | O0-15 | E16-31 | O16-31] so the rope
    # pair swap is a within-32-partition stream shuffle
    sigma = np.zeros(DIM, dtype=np.int64)
    j16 = np.arange(16)
    for h in range(H):
        b = h * 64
        sigma[b + j16] = b + 2 * j16              # E pairs 0-15
        sigma[b + 16 + j16] = b + 2 * j16 + 1     # O pairs 0-15
        sigma[b + 32 + j16] = b + 2 * (j16 + 16)  # E pairs 16-31
        sigma[b + 48 + j16] = b + 2 * (j16 + 16) + 1
    wq_h = np.ascontiguousarray(Wq.T[:, sigma]).astype(BF)
    wk_h = np.ascontiguousarray(Wk.T[:, sigma]).astype(BF)
    wv_h = np.ascontiguousarray(Wv.T).astype(BF)
    wo_h = np.ascontiguousarray(Wo.T).astype(BF)

    # rope tables in sigma row order, sin sign-folded:
    # row r (within 64-row head block): freq f(r), sign -1 on E rows
    rf = np.zeros(64, dtype=np.int64)
    sg = np.zeros(64, dtype=np.float32)
    rf[0:16], sg[0:16] = j16, -1.0          # E pairs 0-15
    rf[16:32], sg[16:32] = j16, 1.0         # O pairs 0-15
    rf[32:48], sg[32:48] = j16 + 16, -1.0   # E pairs 16-31
    rf[48:64], sg[48:64] = j16 + 16, 1.0    # O pairs 16-31
    rf = np.tile(rf, 2)
    sg = np.tile(sg, 2)

    kk = np.arange(P)[:, None]
    qq = np.arange(P)[None, :]
    mko_h = (kk > qq).astype(np.float32).astype(BF)    # keep k > q (oldest)
    mkd_h = (kk <= qq).astype(np.float32).astype(BF)   # keep k <= q (diag)

    inv_freq = 1.0 / (10000.0 ** (np.arange(0, D, 2, dtype=np.float32) / D))
    xT = x2.T  # [DIM, S]

    in_maps = []
    for core in range(NCORES):
        lo = core * SL - W
        xsh = np.zeros((DIM, SK), dtype=np.float32)
        if lo < 0:
            xsh[:, W:] = xT[:, :SL]
        else:
            xsh[:, :] = xT[:, lo:lo + SK]
        pos = np.arange(lo, lo + SK, dtype=np.float32)
        ang = pos[None, :] * inv_freq[rf][:, None]      # [128, SK]
        in_maps.append({
            "xt": xsh.astype(BF),
            "wq": wq_h, "wk": wk_h, "wv": wv_h, "wo": wo_h,
            "ropc": np.ascontiguousarray(np.cos(ang)).astype(BF),
            "rops": np.ascontiguousarray(
                sg[:, None] * np.sin(ang)).astype(BF),
            "vone": np.ascontiguousarray(
                (pos.reshape(NKT, P).T >= 0).astype(np.float32)).astype(BF),
            "mko": mko_h, "mkd": mkd_h,
        })
    return in_maps


def kernel(x, Wq, Wk, Wv, Wo, window_size, _trace=False, _trace_kwargs=None):
    assert int(window_size) == W
    if "nc" not in _compiled:
        _compiled["nc"] = _build()
    nc = _compiled["nc"]
    in_maps = _prep_inputs(np.asarray(x), np.asarray(Wq), np.asarray(Wk),
                           np.asarray(Wv), np.asarray(Wo))
    res = run_bass_kernel_spmd(nc, in_maps, core_ids=list(range(NCORES)),
                               trace=_trace, **(_trace_kwargs or {}))
    outp = np.concatenate([res.results[c]["out"] for c in range(NCORES)],
                          axis=0)
    _compiled["last_result"] = res
    return outp.reshape(1, S, DIM).astype(np.float32)


if __name__ == "__main__":
    np.random.seed(0)
    x = np.random.randn(1, S, DIM).astype(np.float32)
    sd = 1.0 / np.sqrt(DIM)
    ws = [np.random.randn(DIM, DIM).astype(np.float32) * sd for _ in range(4)]
    y = kernel(x, *ws, window_size=W)
    print("kernel output", y.shape, y.dtype, np.abs(y).max())


# revision 30
# speedup vs baseline: 1.2272x; 1.1490x over previous
"""Self-contained Trainium2 Bass kernel for sliding-window attention.

Problem (hardcoded): B=1, S=8192, dim=1024, H=16 heads, D=64 head dim,
window=512, fp32 I/O.  y = (softmax(mask(rope(xWq^T) rope(xWk^T)^T / 8)) xWv^T) Wo^T

Strategy: sequence-parallel over 8 NeuronCores. Each core owns 1024 query
rows and additionally recomputes K/V for the 512-row halo to its left
(core 0's halo is zero-padded and neutralized via a per-core "vones"
column so no collective is needed).  All matmuls run in bf16 (fp32 PSUM
accumulation).

v2 schedule changes vs baseline:
  - priority-ordered fine-grained input DMAs round-robined over 4 queues
    (first-needed tiles land first) + PE warm-up matmuls during the load
    to climb the tensor-engine p-state ramp.
  - rope uses a 16-wide even/odd interleave per head so the pair swap is
    a single DVE stream_shuffle (within-32-partition), plus a signed sin
    table: 1 shuffle + 2 muls + 1 add, all 128-partition-wide.
  - sliding-window causality applied as 0/1 mask multiplies on the DVE
    (post-exp) instead of rank-128 mask matmuls on the PE.
  - per-(head,kt) software pipelining: scores(kt+1) is emitted before
    exp/PV(kt) so the PE streams while the scalar engine runs exp.
  - softmax denominator broadcast via a stride-0 DMA instead of the slow
    gpsimd partition_broadcast.
  - output projection evictions alternate vector/scalar and out-tiles DMA
    immediately on rotating queues (no serial tail).

Layouts (per core):
  xT    [1024(d), 1536(s)]  x^T shard incl. halo (bf16)
  wq/wk [1024(d), 1024(e')] Wq^T / Wk^T with a per-head column permutation
                            (rope de-interleave: head h's rows are
                            [E0-15 | O0-15 | E16-31 | O16-31])
  wv    [1024(d), 1024(e)]  Wv^T (no permutation), wo = Wo^T
  Q^T/K^T are produced in [e', s] layout (weight-stationary matmuls) so
  attention needs no transposes: scores are computed transposed,
  S^T[k, q], the softmax denominator comes free from a ones-column
  appended to V, and PV directly yields o^T[e, q] — the lhsT of the
  output projection.
"""
import sys

sys.path.insert(0, "/opt/trn_rl_repo")

import numpy as np
import ml_dtypes

import concourse.bass as bass
import concourse.mybir as mybir
from concourse import bacc
from concourse.tile import TileContext
from concourse.bass_utils import run_bass_kernel_spmd

BF = ml_dtypes.bfloat16
NCORES = 8
S, DIM, H, D, W = 8192, 1024, 16, 64, 512
SL = S // NCORES          # 1024 own rows / core
SK = SL + W               # 1536 rows incl. left halo
P = 128
NKT = SK // P             # 12 kv tiles
NQB = SL // P             # 8 query tiles
dt = mybir.dt

_compiled = {}

SWAP16 = list(range(16, 32)) + list(range(16))  # within-32 E<->O block swap
VA = 80   # V_aug columns: 64 V + 1 ones + pad (32B-aligned stride)


def _build():
    nc = bacc.Bacc("TRN2", target_bir_lowering=False, debug=False,
                   num_devices=NCORES)
    def param(name, shape, dtype=dt.bfloat16, out=False):
        return nc.declare_dram_parameter(name, shape, dtype, isOutput=out)

    xt = param("xt", [DIM, SK])
    wq = param("wq", [DIM, DIM])
    wk = param("wk", [DIM, DIM])
    wv = param("wv", [DIM, DIM])
    wo = param("wo", [DIM, DIM])
    ropc = param("ropc", [P, SK])
    rops = param("rops", [P, SK])       # sign-folded sin table
    vone = param("vone", [P, NKT])      # host-expanded per-kt validity
    mko = param("mko", [P, P])          # keep-mask, oldest block  [k, q]
    mkd = param("mkd", [P, P])          # keep-mask, diagonal block [k, q]
    out = param("out", [SL, DIM], dt.float32, out=True)

    with TileContext(nc) as tc:
        _body(nc, tc, xt, wq, wk, wv, wo, ropc, rops, vone, mko, mkd, out)
    nc.compile()
    return nc


def _brd2(ap_slice, n):
    """Insert a stride-0 middle free dim of size n into a [p, c] AP."""
    return bass.AP(tensor=ap_slice.tensor, offset=ap_slice.offset,
                   ap=[ap_slice.ap[0], [0, n], ap_slice.ap[1]])


def _body(nc, tc, xt, wq, wk, wv, wo, ropc, rops, vone, mko, mkd, out):
    f32, bf16 = dt.float32, dt.bfloat16

    with tc.tile_pool(name="persist", bufs=1) as per:
        # long-lived SBUF tensors
        v_sb = per.tile([P, NKT, H, VA], bf16)    # V_aug: [V(64)|ones|pad]
        qt_sb = per.tile([P, 8, SL], bf16)        # Q^T (rope'd, sigma layout)
        kt_sb = per.tile([P, 8, SK], bf16)        # K^T
        ot_sb = per.tile([P, 8, SL], bf16)        # o^T (normalized)
        ropc_sb = per.tile([P, SK], bf16)
        rops_sb = per.tile([P, SK], bf16)
        mko_sb = per.tile([P, P], bf16)
        mkd_sb = per.tile([P, P], bf16)
        vone_sb = per.tile([P, NKT], bf16)

        with tc.tile_pool(name="xtp", bufs=1) as xtp, \
             tc.tile_pool(name="proj", bufs=1, space="PSUM") as projp, \
             tc.tile_pool(name="raw", bufs=3) as rawp, \
             tc.tile_pool(name="swp", bufs=3) as swp, \
             tc.tile_pool(name="mcp", bufs=2) as mcp, \
             tc.tile_pool(name="msp", bufs=2) as msp, \
             tc.tile_pool(name="wqk", bufs=1) as wqkp, \
             tc.tile_pool(name="pt", bufs=3) as ptp, \
             tc.tile_pool(name="st", bufs=3, space="PSUM") as stp, \
             tc.tile_pool(name="st5", bufs=1, space="PSUM") as st5p, \
             tc.tile_pool(name="pv", bufs=3, space="PSUM") as pvp, \
             tc.tile_pool(name="osb", bufs=2) as osbp, \
             tc.tile_pool(name="eps", bufs=1) as epsp, \
             tc.tile_pool(name="bcp", bufs=1) as bcp:
            xt_sb = xtp.tile([P, 8, SK], bf16)
            wk_sb = wqkp.tile([P, 8, DIM], bf16)
            wq_sb = wqkp.tile([P, 8, DIM], bf16)
            wv_sb = wqkp.tile([P, 8, DIM], bf16)
            wo_sb = wqkp.tile([P, 8, DIM], bf16)

            xt_r = xt.ap().rearrange("(d p) s -> p d s", p=P)
            wk_r = wk.ap().rearrange("(d p) e -> p d e", p=P)
            wq_r = wq.ap().rearrange("(d p) e -> p d e", p=P)
            wv_r = wv.ap().rearrange("(d p) e -> p d e", p=P)
            wo_r = wo.ap().rearrange("(e p) n -> p e n", p=P)

            # ---- priority-ordered input DMAs, round-robin over 3 queues
            # (each queue drains in order, so each queue's prefix is the
            # first-needed data; ~0.65us issue cost per DMA dominates below
            # ~256KB, so pieces are kept large) ----
            dmas = [
                (ropc_sb[:, :], ropc[:, :]),
                (rops_sb[:, :], rops[:, :]),
                (mko_sb[:, :], mko[:, :]),
                (mkd_sb[:, :], mkd[:, :]),
                (vone_sb[:, :], vone[:, :]),
            ]
            for d in range(8):
                dmas.append((xt_sb[:, d, 0:768], xt_r[:, d, 0:768]))
            for d in range(8):
                dmas.append((wk_sb[:, d, :], wk_r[:, d, :]))
            for d in range(8):
                dmas.append((xt_sb[:, d, 768:1536], xt_r[:, d, 768:1536]))
            for d in range(8):
                dmas.append((wq_sb[:, d, :], wq_r[:, d, :]))
            for d in range(8):
                dmas.append((wv_sb[:, d, :], wv_r[:, d, :]))
            for d in range(8):
                dmas.append((wo_sb[:, d, :], wo_r[:, d, :]))
            # first waves may use the scalar queue (it is idle until the
            # first projection eviction ~12us in); later waves must not sit
            # ahead of evictions/exps on the scalar queue
            DQ3 = [nc.sync, nc.scalar, nc.gpsimd]
            DQ2 = [nc.sync, nc.gpsimd]
            for i, (o_ap, i_ap) in enumerate(dmas):
                if i < 30:
                    DQ3[i % 3].dma_start(out=o_ap, in_=i_ap)
                else:
                    DQ2[i % 2].dma_start(out=o_ap, in_=i_ap)

            # ---- PE warm-up: dummy matmuls on the rope table while the
            # real inputs stream in, to climb the p-state ramp ----
            warm_ps = projp.tile([P, 512], f32, name="warm", tag="ps")
            for _ in range(8):
                nc.tensor.matmul(warm_ps, lhsT=ropc_sb[:, 0:P],
                                 rhs=ropc_sb[:, 0:512], start=True, stop=True)

            # ones column of V_aug (per-core halo-validity mask)
            for kt_i in range(NKT):
                nc.vector.tensor_copy(
                    out=v_sb[:, kt_i, :, 64],
                    in_=_brd2(vone_sb[:, kt_i:kt_i + 1], H))

            def rope_chunk(raw, dst, c0):
                # raw [P, 512] bf16 -> dst [P, 512] (rope'd).
                # dst = raw*cos + swap16(raw)*sin_signed
                cseg = slice(c0, c0 + 512)
                rsw = swp.tile([P, 512], bf16, name="rsw", tag="rsw")
                mcos = mcp.tile([P, 512], bf16, name="mcos", tag="mcos")
                msw = msp.tile([P, 512], bf16, name="msw", tag="msw")
                nc.vector.stream_shuffle(rsw, raw, SWAP16)
                nc.vector.tensor_mul(mcos, raw, ropc_sb[:, cseg])
                nc.vector.tensor_mul(msw, rsw, rops_sb[:, cseg])
                nc.vector.tensor_add(dst, mcos, msw)

            def proj_rope_et(w_sb, dst, s0, et, mid_hook=None):
                # dst[:, et, s0:SK] = rope((W^T)^T @ xT[:, s0:SK])
                for si, sp in enumerate(range(s0, SK, 512)):
                    ps = projp.tile([P, 512], f32, name="ps", tag="ps")
                    for d in range(8):
                        nc.tensor.matmul(
                            ps,
                            lhsT=w_sb[:, d, et * P:(et + 1) * P],
                            rhs=xt_sb[:, d, sp:sp + 512],
                            start=(d == 0), stop=(d == 7))
                    if si == 0 and mid_hook is not None:
                        mid_hook()
                    raw = rawp.tile([P, 512], bf16, name="raw", tag="raw")
                    nc.scalar.copy(out=raw, in_=ps)
                    rope_chunk(raw, dst[:, et, sp - s0:sp - s0 + 512], sp)

            # ---- attention with cross-(head,kt) software pipelining ----
            pv_state = {}     # h -> [pv_t_g0, pv_t_g1]
            pend = []         # [(h, kt, st_ps, lo, hi)]
            step = [0]        # global post counter
            deferred = []     # [(due_step, fn)] late-emitted retire tails

            def emit_scores(h, kt):
                et, hr = h // 2, (h % 2) * 64
                lo, hi = max(kt - 4, 0), min(kt, 7)
                nqb = hi - lo + 1
                n0 = min(nqb, 4) * P
                kh = kt_sb[hr:hr + 64, et, kt * P:(kt + 1) * P]
                st_ps = stp.tile([P, 512], f32, name="st_ps")
                st5 = None
                nc.tensor.matmul(
                    st_ps[:, 0:n0], lhsT=kh,
                    rhs=qt_sb[hr:hr + 64, et, lo * P:lo * P + n0],
                    start=True, stop=True)
                if nqb == 5:
                    st5 = st5p.tile([P, P], f32, name="st5")
                    nc.tensor.matmul(
                        st5, lhsT=kh,
                        rhs=qt_sb[hr:hr + 64, et, (lo + 4) * P:(lo + 5) * P],
                        start=True, stop=True)
                pend.append((h, kt, st_ps, st5, lo, hi))

            def emit_post():
                if not pend:
                    return
                h, kt, st_ps, st5, lo, hi = pend.pop(0)
                et, hr = h // 2, (h % 2) * 64
                nqb = hi - lo + 1
                n0 = min(nqb, 4) * P
                pv_t = pv_state.setdefault(h, [None, None])
                p_t = ptp.tile([P, 640], bf16, name="p_t")
                nc.scalar.activation(
                    out=p_t[:, 0:n0], in_=st_ps[:, 0:n0],
                    func=mybir.ActivationFunctionType.Exp, scale=0.125)
                if st5 is not None:
                    nc.scalar.activation(
                        out=p_t[:, 512:640], in_=st5,
                        func=mybir.ActivationFunctionType.Exp, scale=0.125)
                # mask-muls run on the (otherwise idle) Pool engine so the
                # PV critical path never sits behind vector-queue parks
                if kt >= 4:          # causal (diagonal) block sits at col 0
                    nc.gpsimd.tensor_mul(p_t[:, 0:P], p_t[:, 0:P], mkd_sb)
                if kt <= 7:          # oldest block sits at col kt-lo
                    c = (kt - lo) * P
                    nc.gpsimd.tensor_mul(p_t[:, c:c + P], p_t[:, c:c + P],
                                         mko_sb)
                # PV: one matmul per touched pv bank, batched over qbs.
                for g in (0, 1):
                    c0, c1 = max(lo, 4 * g), min(hi, 4 * g + 3)
                    if c0 > c1:
                        continue
                    if pv_t[g] is None:
                        pv_t[g] = pvp.tile([P, 512], f32, name="pvt",
                                           tag="pvt")
                    nc.tensor.matmul(
                        pv_t[g][0:VA, (c0 % 4) * P:(c1 % 4 + 1) * P],
                        lhsT=v_sb[:, kt, h, :],
                        rhs=p_t[:, (c0 - lo) * P:(c1 - lo + 1) * P],
                        start=(kt == 4 * g), stop=(kt == 4 * g + 7),
                        skip_group_check=True)
                for g in (0, 1):
                    if kt == 4 * g + 7:
                        # retire: normalize by the ones-row denominator.
                        # reciprocal runs lane-parallel via a DMA reshape;
                        # the [64,512] broadcast is a stride-0 DMA.  Every
                        # hop is emitted one pipeline step after its
                        # producer so no queue ever parks on the chain
                        # (a parked vector queue would block the mask-muls
                        # the PV matmuls need).
                        pv = pv_t[g]
                        rrow = epsp.tile([1, 512], f32, name="rrow",
                                         tag="rrow")
                        rcs = epsp.tile([128, 4], f32, name="rcs", tag="rcs")
                        rcr = epsp.tile([128, 4], f32, name="rcr", tag="rcr")
                        rc = epsp.tile([1, 512], f32, name="rc", tag="rc")
                        bc = bcp.tile([64, 512], f32, name="bc", tag="bc")
                        nc.scalar.copy(out=rrow, in_=pv[64:65, :])
                        dst = ot_sb[hr:hr + 64, et, g * 512:(g + 1) * 512]
                        s0 = step[0]
                        deferred.append((s0 + 1, lambda rcs=rcs, rrow=rrow:
                                         nc.sync.dma_start(out=rcs,
                                                           in_=rrow)))
                        def mk2(rcr=rcr, rcs=rcs, rc=rc):
                            nc.vector.reciprocal(rcr, rcs)
                            nc.sync.dma_start(out=rc, in_=rcr)
                        deferred.append((s0 + 2, mk2))
                        deferred.append((s0 + 3, lambda bc=bc, rc=rc:
                                         nc.sync.dma_start(
                                             out=bc,
                                             in_=_brd2(rc[0:1, :], 64))))
                        deferred.append((s0 + 4, lambda dst=dst, pv=pv,
                                         bc=bc:
                                         nc.vector.tensor_mul(
                                             dst, pv[0:64, :], bc)))
                step[0] += 1
                for due, fn in [x for x in deferred]:
                    if due <= step[0]:
                        fn()
                        deferred.remove((due, fn))

            def emit_att(h):
                for kt in range(NKT):
                    emit_scores(h, kt)
                    if len(pend) > 2:
                        emit_post()

            def flush_pend():
                while pend:
                    emit_post()
                for due, fn in deferred:
                    fn()
                deferred.clear()

            # ---- projections for the first two etile pairs ----
            proj_rope_et(wk_sb, kt_sb, 0, 0)
            proj_rope_et(wq_sb, qt_sb, W, 0)
            proj_rope_et(wk_sb, kt_sb, 0, 1)
            proj_rope_et(wq_sb, qt_sb, W, 1)

            # ---- V projection (scalar/vector engines are busy with
            # rope+exp meanwhile) ----
            for st_i in range(NKT):
                for eh in range(2):
                    ps = projp.tile([P, 512], f32, name="psv", tag="ps")
                    for d in range(8):
                        nc.tensor.matmul(
                            ps,
                            lhsT=xt_sb[:, d, st_i * P:(st_i + 1) * P],
                            rhs=wv_sb[:, d, eh * 512:(eh + 1) * 512],
                            start=(d == 0), stop=(d == 7))
                    # scatter heads into V_aug slots [st, h, 0:64]
                    nc.scalar.copy(
                        out=v_sb[:, st_i, eh * 8:(eh + 1) * 8, 0:64],
                        in_=ps[:, :].rearrange("p (h e) -> p h e", h=8))

            for et in range(8):
                emit_att(2 * et)
                emit_att(2 * et + 1)
                if et + 2 < 8:
                    proj_rope_et(wk_sb, kt_sb, 0, et + 2,
                                 mid_hook=flush_pend)
                    proj_rope_et(wq_sb, qt_sb, W, et + 2)

            # ---- output projection, out-tiles DMA'd as they finish ----
            flush_pend()
            for qt_i in range(NQB):
                for nh in range(2):
                    ps = pvp.tile([P, 512], f32, name="pso", tag="pvt")
                    for p in range(8):
                        nc.tensor.matmul(
                            ps,
                            lhsT=ot_sb[:, p, qt_i * P:(qt_i + 1) * P],
                            rhs=wo_sb[:, p, nh * 512:(nh + 1) * 512],
                            start=(p == 0), stop=(p == 7))
                    o_sb = osbp.tile([P, 512], f32, name="o_sb")
                    if nh == 0:
                        nc.vector.tensor_copy(o_sb, ps)
                    else:
                        nc.scalar.copy(out=o_sb, in_=ps)
                    DQ3[(qt_i * 2 + nh) % 3].dma_start(
                        out=out[qt_i * P:(qt_i + 1) * P,
                                nh * 512:(nh + 1) * 512],
                        in_=o_sb)


def _prep_inputs(x, Wq, Wk, Wv, Wo):
    """Host-side shard/layout prep -> list of 8 per-core input dicts."""
    x2 = np.ascontiguousarray(x.reshape(S, DIM).astype(np.float32))
    # head-row permutation: [E0-15 | O0-15 | E16-31 | O16-31] so the rope
    # pair swap is a within-32-partition stream shuffle
    sigma = np.zeros(DIM, dtype=np.int64)
    j16 = np.arange(16)
    for h in range(H):
        b = h * 64
        sigma[b + j16] = b + 2 * j16              # E pairs 0-15
        sigma[b + 16 + j16] = b + 2 * j16 + 1     # O pairs 0-15
        sigma[b + 32 + j16] = b + 2 * (j16 + 16)  # E pairs 16-31
        sigma[b + 48 + j16] = b + 2 * (j16 + 16) + 1
    wq_h = np.ascontiguousarray(Wq.T[:, sigma]).astype(BF)
    wk_h = np.ascontiguousarray(Wk.T[:, sigma]).astype(BF)
    wv_h = np.ascontiguousarray(Wv.T).astype(BF)
    wo_h = np.ascontiguousarray(Wo.T).astype(BF)

    # rope tables in sigma row order, sin sign-folded:
    # row r (within 64-row head block): freq f(r), sign -1 on E rows
    rf = np.zeros(64, dtype=np.int64)
    sg = np.zeros(64, dtype=np.float32)
    rf[0:16], sg[0:16] = j16, -1.0          # E pairs 0-15
    rf[16:32], sg[16:32] = j16, 1.0         # O pairs 0-15
    rf[32:48], sg[32:48] = j16 + 16, -1.0   # E pairs 16-31
    rf[48:64], sg[48:64] = j16 + 16, 1.0    # O pairs 16-31
    rf = np.tile(rf, 2)
    sg = np.tile(sg, 2)

    kk = np.arange(P)[:, None]
    qq = np.arange(P)[None, :]
    mko_h = (kk > qq).astype(np.float32).astype(BF)    # keep k > q (oldest)
    mkd_h = (kk <= qq).astype(np.float32).astype(BF)   # keep k <= q (diag)

    inv_freq = 1.0 / (10000.0 ** (np.arange(0, D, 2, dtype=np.float32) / D))
    xT = x2.T  # [DIM, S]

    in_maps = []
    for core in range(NCORES):
        lo = core * SL - W
        xsh = np.zeros((DIM, SK), dtype=np.float32)
        if lo < 0:
            xsh[:, W:] = xT[:, :SL]
        else:
            xsh[:, :] = xT[:, lo:lo + SK]
        pos = np.arange(lo, lo + SK, dtype=np.float32)
        ang = pos[None, :] * inv_freq[rf][:, None]      # [128, SK]
        in_maps.append({
            "xt": xsh.astype(BF),
            "wq": wq_h, "wk": wk_h, "wv": wv_h, "wo": wo_h,
            "ropc": np.ascontiguousarray(np.cos(ang)).astype(BF),
            "rops": np.ascontiguousarray(
                sg[:, None] * np.sin(ang)).astype(BF),
            "vone": np.ascontiguousarray(
                (pos.reshape(NKT, P).T >= 0).astype(np.float32)).astype(BF),
            "mko": mko_h, "mkd": mkd_h,
        })
    return in_maps


def kernel(x, Wq, Wk, Wv, Wo, window_size, _trace=False, _trace_kwargs=None):
    assert int(window_size) == W
    if "nc" not in _compiled:
        _compiled["nc"] = _build()
    nc = _compiled["nc"]
    in_maps = _prep_inputs(np.asarray(x), np.asarray(Wq), np.asarray(Wk),
                           np.asarray(Wv), np.asarray(Wo))
    res = run_bass_kernel_spmd(nc, in_maps, core_ids=list(range(NCORES)),
                               trace=_trace, **(_trace_kwargs or {}))
    outp = np.concatenate([res.results[c]["out"] for c in range(NCORES)],
                          axis=0)
    _compiled["last_result"] = res
    return outp.reshape(1, S, DIM).astype(np.float32)


if __name__ == "__main__":
    np.random.seed(0)
    x = np.random.randn(1, S, DIM).astype(np.float32)
    sd = 1.0 / np.sqrt(DIM)
    ws = [np.random.randn(DIM, DIM).astype(np.float32) * sd for _ in range(4)]
    y = kernel(x, *ws, window_size=W)
    print("kernel output", y.shape, y.dtype, np.abs(y).max())
